# revision 31
# baseline (speedup 1.0000x reference)
"""Multi-head attention (B=4, T=2048, D=1024, H=16, causal) on 8 trn2 cores.

Sharding: 2 heads per core (tensor-parallel on H). Each core computes
q/k/v projections for its 128-row slice of Wq/Wk/Wv over all tokens,
causal attention for its 2 heads over all 4 batches, and a partial
o_proj contribution y_c = A_c @ Wo[:, slice].T.  The host sums the 8
partial outputs (the "all-reduce after o_proj" of the sharding hint).

Layout strategy: activations live transposed ([feature, token]) on
device so the matmul contraction dim is always the partition dim:
  qT/kT/vT [128=2*64, T]  <- W-slice-T tiles @ xT chunks
  scoresT  [128 k-tok, 512 q-tok] <- kT-tile.T @ qT   (per head)
  softmax: no max-subtraction (logits are O(+-8); exp is safe in f32);
           exp on ACT; causal masking multiplies diagonal tiles with a
           precomputed 0/1 band; row sums come free as a column of
           ones appended to v; normalization is deferred to after
           attn@v: broadcast the denominators across partitions with
           K=1 PE outer products, reciprocal_approx on the broadcast.
  aT       [65, 512] PSUM accum over k-tiles (row 64 = softmax denom)
  y        [tok, D] partial, via lhsT=aT tiles (natural output layout).
Matmuls run in bf16 (1 cyc/row on the PE; f32r measured 2 cyc/row).
"""

import sys

sys.path.insert(0, "/opt/trn_rl_repo")

import ml_dtypes
import numpy as np

import concourse.bass as bass
import concourse.mybir as mybir
import concourse.tile as tile
from concourse.masks import make_identity

F32 = mybir.dt.float32
F32R = mybir.dt.float32r
BF16 = mybir.dt.bfloat16
EXP = mybir.ActivationFunctionType.Exp

B, T, D, H = 4, 2048, 1024, 16
DH = D // H
NCORES = 8
HPC = H // NCORES          # heads per core (2)
HS = HPC * DH              # head-slice width per core (128)
QC = 512                   # q-tokens per chunk (PSUM free-dim limit, f32)
KT = 128                   # k-tokens per tile (partition dim)

_wsplit_n = [0]


def split_embedded_waits(nc):
    """Move embedded on_wait conditions into standalone EventSemaphore
    instructions.  The walrus build in this container rejects embedded
    sync waits on compute instruction structs ("Too many sync wait
    commands"); raw-bass-style standalone waits compile fine."""
    moved = 0
    for func in nc.m.functions:
        for blk in func.blocks:
            insts = list(blk.instructions)
            out = []
            changed = False
            for ins in insts:
                si = ins.sync_info
                waits = list(si.on_wait) if (si is not None and si.on_wait) else []
                limit = 1 if ins.opcode in ("EventSemaphore", "Drain") else 0
                if len(waits) > limit:
                    head = waits[:-limit] if limit else waits
                    tail = waits[-limit:] if limit else []
                    for w in head:
                        _wsplit_n[0] += 1
                        out.append(
                            mybir.InstEventSemaphore(
                                name=f"I-wsplit-{_wsplit_n[0]}",
                                engine=ins.engine,
                                sync_info=mybir.SyncInfo(on_wait=[w], on_update=[]),
                            )
                        )
                        moved += 1
                    ins.sync_info = mybir.SyncInfo(
                        on_wait=list(tail),
                        on_update=list(si.on_update) if si.on_update else [],
                    )
                    changed = True
                out.append(ins)
            if changed:
                blk.instructions = out
    return moved


def build_nc(nb=B, t=T, split_waits=True):
    """Build the per-core Bass program. nb/t shrinkable for simulation.
    split_waits must be True for hardware (walrus); False for CoreSim."""
    nqc = t // QC            # q-chunks per batch
    nkt = t // KT            # k-tiles per batch
    ntok = nb * t

    nc = bass.Bass("TRN2", target_bir_lowering=False)

    xT_d = nc.dram_tensor("xT", [D, ntok], BF16, kind="ExternalInput")
    wq_d = nc.dram_tensor("wq", [D, HS], BF16, kind="ExternalInput")
    wk_d = nc.dram_tensor("wk", [D, HS], BF16, kind="ExternalInput")
    wv_d = nc.dram_tensor("wv", [D, HS], BF16, kind="ExternalInput")
    wo_d = nc.dram_tensor("wo", [HS, D], BF16, kind="ExternalInput")
    tri_d = nc.dram_tensor("tri", [KT, KT], BF16, kind="ExternalInput")
    y_d = nc.dram_tensor("y", [ntok, D], F32, kind="ExternalOutput")

    nd = D // 128            # 8 d-in tiles

    with tile.TileContext(nc) as tc:
        with (
            tc.tile_pool(name="wpool", bufs=1) as wpool,
            tc.tile_pool(name="xin", bufs=2) as xin,
            tc.tile_pool(name="qkv", bufs=2) as qkvp,
            tc.tile_pool(name="vaug", bufs=2) as vaugp,
            tc.tile_pool(name="exps", bufs=4) as expp,
            tc.tile_pool(name="misc", bufs=2) as miscp,
            tc.tile_pool(name="yout", bufs=3) as youtp,
            tc.tile_pool(name="psc", bufs=2, space="PSUM") as psc,
            tc.tile_pool(name="pa", bufs=1, space="PSUM") as pa,
            tc.tile_pool(name="pm", bufs=2, space="PSUM") as pm,
        ):
            # resident weights
            wq_sb = wpool.tile([128, nd, HS], BF16, tag="wq")
            wk_sb = wpool.tile([128, nd, HS], BF16, tag="wk")
            wv_sb = wpool.tile([128, nd, HS], BF16, tag="wv")
            for wd, ws in ((wq_d, wq_sb), (wk_d, wk_sb), (wv_d, wv_sb)):
                nc.sync.dma_start(ws[:], wd[:].rearrange("(a p) m -> p a m", p=128))
            wo_sb = wpool.tile([HS, D], BF16, tag="wo")
            nc.sync.dma_start(wo_sb[:], wo_d[:])
            tri_sb = wpool.tile([KT, KT], BF16, tag="tri")
            nc.sync.dma_start(tri_sb[:], tri_d[:])
            # constants: f32 masters, rounded into matmul dtypes via copies
            ones_f = wpool.tile([128, max(nkt, 128)], F32, tag="ones_f")
            nc.vector.memset(ones_f[:], 1.0)
            zeros_f = wpool.tile([1, 64], F32, tag="zeros_f")
            nc.vector.memset(zeros_f[:], 0.0)
            # head-expander rows: e0row = [1]*64+[0]*64, e1row = [0]*64+[1]*64
            e0row = wpool.tile([1, 128], F32R, tag="e0row")
            e1row = wpool.tile([1, 128], F32R, tag="e1row")
            nc.vector.tensor_copy(e0row[:, 0:64], ones_f[0:1, 0:64])
            nc.vector.tensor_copy(e0row[:, 64:128], zeros_f[:])
            nc.vector.tensor_copy(e1row[:, 0:64], zeros_f[:])
            nc.vector.tensor_copy(e1row[:, 64:128], ones_f[0:1, 0:64])
            ident = wpool.tile([128, 128], F32, tag="ident")
            make_identity(nc, ident[:])

            class OproJob:
                """Deferred o_proj for one 512-token chunk: 8 matmul+copy
                pieces stepped one at a time between k-iterations, then one
                fused DMA of the [512, D] result."""

                def __init__(self, row0, aT):
                    self.row0, self.aT = row0, aT
                    self.ysb = youtp.tile([128, QC // 128, D], F32, tag="ysb")
                    self.pieces = [
                        (tt, do)
                        for tt in range(QC // 128)
                        for do in range(D // 512)
                    ]
                    self.i = 0

                def step(self):
                    tt, do = self.pieces[self.i]
                    self.i += 1
                    yp = pm.tile([128, 512], F32, tag="m", bufs=1)
                    nc.tensor.matmul(
                        yp[:],
                        self.aT[:, 128 * tt : 128 * (tt + 1)],
                        wo_sb[:, 512 * do : 512 * (do + 1)],
                        start=True,
                        stop=True,
                    )
                    nc.vector.tensor_copy(
                        self.ysb[:, tt, 512 * do : 512 * (do + 1)], yp[:]
                    )
                    if self.i == len(self.pieces):
                        nc.sync.dma_start(
                            y_d[self.row0 : self.row0 + QC, :].rearrange(
                                "(a p) n -> p a n", p=128
                            ),
                            self.ysb[:],
                        )
                        return False
                    return True

            ojob = [None]
            pending_fin = [None]

            def load_xt(b, nsplit=1):
                base = b * t
                xt = xin.tile([128, nd, t], BF16, tag="xt")
                step = t // nsplit
                for s in range(nsplit):
                    nc.sync.dma_start(
                        xt[:, :, s * step : (s + 1) * step],
                        xT_d[:, base + s * step : base + (s + 1) * step].rearrange(
                            "(a p) n -> p a n", p=128
                        ),
                    )
                return xt

            

            xt_next = load_xt(0, nsplit=4)
            for b in range(nb):
                base = b * t
                xt_b = xt_next
                # ---- q/k/v projections for batch b ----
                # kT is stored zero-padded per head (kT0z rows 0:64 = head0,
                # rows 64:128 = 0; kT1z the reverse) so the scores matmul can
                # stream the full 128-partition qT at full SBUF rate.
                qT = qkvp.tile([128, t], BF16, tag="qT")
                kT0z = qkvp.tile([128, t], BF16, tag="kT0z")
                kT1z = qkvp.tile([128, t], BF16, tag="kT1z")
                nc.vector.memset(kT0z[64:128, :], 0.0)
                nc.vector.memset(kT1z[0:64, :], 0.0)
                vT = qkvp.tile([128, t], F32, tag="vT")
                for ch in range(nqc):
                    cs = slice(QC * ch, QC * (ch + 1))
                    for wi, ws in enumerate((wq_sb, wk_sb, wv_sb)):
                        ps = psc.tile([128, 2, QC], F32, tag="sc")
                        for kd in range(nd):
                            nc.tensor.matmul(
                                ps[:, 0, :],
                                ws[:, kd, :],
                                xt_b[:, kd, cs],
                                start=(kd == 0),
                                stop=(kd == nd - 1),
                            )
                        if wi == 0:
                            nc.vector.tensor_copy(qT[:, cs], ps[:, 0, :])
                        elif wi == 1:
                            nc.vector.tensor_copy(kT0z[0:64, cs], ps[0:64, 0, :])
                            nc.vector.tensor_copy(kT1z[64:128, cs], ps[64:128, 0, :])
                        else:
                            nc.vector.tensor_copy(vT[:, cs], ps[:, 0, :])

                if b + 1 < nb:
                    xt_next = load_xt(b + 1)

                # ---- transpose v into [k-tok, dh(+ones)] tiles ----
                v0 = vaugp.tile([128, nkt, DH + 1], BF16, tag="v0")
                v1 = vaugp.tile([128, nkt, DH + 1], BF16, tag="v1")
                nc.vector.tensor_copy(v0[:, :, DH : DH + 1], ones_f[:, 0:nkt])
                nc.vector.tensor_copy(v1[:, :, DH : DH + 1], ones_f[:, 0:nkt])
                for kt in range(nkt):
                    tp = pm.tile([128, 512], F32, tag="rb", bufs=1)
                    nc.tensor.transpose(
                        tp[:, 0:128], vT[:, KT * kt : KT * (kt + 1)], ident[:]
                    )
                    nc.vector.tensor_copy(v0[:, kt, 0:DH], tp[:, 0:DH])
                    nc.vector.tensor_copy(v1[:, kt, 0:DH], tp[:, DH : 2 * DH])

                # ---- attention + o_proj per q-chunk ----
                # k-loop emitted software-pipelined (scores two steps ahead
                # of attn@v); the previous chunk's o_proj matmuls and output
                # copies are drip-fed between k-iterations so the PE never
                # sits in a blocked o_proj stretch, and normalization is a
                # GPSIMD divide by the PE-broadcast denominators (no
                # reciprocal, nothing heavy on the DVE critical path).
                for qc in range(nqc):
                    q0 = QC * qc
                    apair = pa.tile([DH + 1, 2, QC], F32, tag="apair")
                    hi = qc * (QC // KT) + (QC // KT)  # causal: k-tiles 0..hi-1

                    def emit_scores(kt):
                        scp = psc.tile([128, 2, QC], F32, tag="sc")
                        nc.tensor.matmul(
                            scp[:, 0, :],
                            kT0z[:, KT * kt : KT * (kt + 1)],
                            qT[:, q0 : q0 + QC],
                            start=True,
                            stop=True,
                        )
                        nc.tensor.matmul(
                            scp[:, 1, :],
                            kT1z[:, KT * kt : KT * (kt + 1)],
                            qT[:, q0 : q0 + QC],
                            start=True,
                            stop=True,
                        )
                        return scp

                    def emit_tail(kt, scp):
                        ep = expp.tile([128, 2, QC], BF16, tag="ep")
                        off = KT * kt - q0
                        if off < 0:
                            nc.scalar.activation(ep[:], scp[:], EXP)
                        else:
                            # diagonal tile: exp only the causally valid
                            # span, zero the rest, apply the 0/1 band
                            for h in (0, 1):
                                nc.scalar.activation(
                                    ep[:, h, off:QC], scp[:, h, off:QC], EXP
                                )
                                if off > 0:
                                    nc.vector.memset(ep[:, h, 0:off], 0.0)
                                nc.vector.tensor_mul(
                                    ep[:, h, off : off + KT],
                                    ep[:, h, off : off + KT],
                                    tri_sb[:],
                                )
                        for h, vh in ((0, v0), (1, v1)):
                            nc.tensor.matmul(
                                apair[:, h, :],
                                vh[:, kt, :],
                                ep[:, h, :],
                                start=(kt == 0),
                                stop=(kt == hi - 1),
                                skip_group_check=True,
                            )

                    pend = [emit_scores(0)]
                    if hi > 1:
                        pend.append(emit_scores(1))
                    if pending_fin[0] is not None:
                        pending_fin[0]()
                        pending_fin[0] = None
                    for kt in range(2, hi):
                        emit_tail(kt - 2, pend.pop(0))
                        pend.append(emit_scores(kt))
                        if ojob[0] is not None and not ojob[0].step():
                            ojob[0] = None
                    for j, scp in enumerate(pend):
                        emit_tail(hi - len(pend) + j, scp)
                    while ojob[0] is not None:
                        if not ojob[0].step():
                            ojob[0] = None

                    # free apair fast: pull out the two heads + denominators
                    sums01 = miscp.tile([1, 2, QC], F32R, tag="sums01")
                    nc.vector.tensor_copy(sums01[:], apair[DH : DH + 1, :, :])
                    aT = qkvp.tile([128, QC], BF16, tag="aT", bufs=3)
                    nc.scalar.copy(aT[0:DH, :], apair[0:DH, 0, :])
                    nc.scalar.copy(aT[DH : 2 * DH, :], apair[0:DH, 1, :])
                    def fin(sums01=sums01, aT=aT, row0=base + q0):
                        # broadcast denominators across partitions (K=1 PE
                        # outer products), reciprocal, scale, queue o_proj
                        rb = pm.tile([128, 512], F32, tag="rb", bufs=1)
                        nc.tensor.matmul(
                            rb[:, 0:QC], e0row[:], sums01[:, 0, :],
                            start=True, stop=False, skip_group_check=True,
                        )
                        nc.tensor.matmul(
                            rb[:, 0:QC], e1row[:], sums01[:, 1, :],
                            start=False, stop=True, skip_group_check=True,
                        )
                        rcp = miscp.tile([128, QC], F32, tag="rcp")
                        nc.vector.reciprocal(rcp[:], rb[:, 0:QC])
                        nc.vector.tensor_mul(aT[:], aT[:], rcp[:])
                        ojob[0] = OproJob(row0, aT)

                    pending_fin[0] = fin
            if pending_fin[0] is not None:
                pending_fin[0]()
                pending_fin[0] = None
            while ojob[0] is not None:
                if not ojob[0].step():
                    ojob[0] = None

    if split_waits:
        split_embedded_waits(nc)
    return nc


def make_tri():
    tri = np.zeros((KT, KT), np.float32)
    j = np.arange(KT)[None, :]
    k = np.arange(KT)[:, None]
    tri[j >= k] = 1.0
    return tri.astype(ml_dtypes.bfloat16)


def make_in_maps(x, Wq, Wk, Wv, Wo):
    ntok = x.shape[0] * x.shape[1]
    bf = ml_dtypes.bfloat16
    xT = np.ascontiguousarray(x.reshape(ntok, D).T).astype(bf)
    tri = make_tri()
    scale = np.float32(1.0 / np.sqrt(DH))
    in_maps = []
    for c in range(NCORES):
        hs = slice(HS * c, HS * (c + 1))
        in_maps.append(
            {
                "xT": xT,
                "wq": np.ascontiguousarray((Wq[hs, :] * scale).T).astype(bf),
                "wk": np.ascontiguousarray(Wk[hs, :].T).astype(bf),
                "wv": np.ascontiguousarray(Wv[hs, :].T).astype(bf),
                "wo": np.ascontiguousarray(Wo[:, hs].T).astype(bf),
                "tri": tri,
            }
        )
    return in_maps


_NC = None


def kernel(**inputs):
    global _NC
    x = np.asarray(inputs["x"], np.float32)
    Wq = np.asarray(inputs["Wq"], np.float32)
    Wk = np.asarray(inputs["Wk"], np.float32)
    Wv = np.asarray(inputs["Wv"], np.float32)
    Wo = np.asarray(inputs["Wo"], np.float32)

    from concourse.bass_utils import run_bass_kernel_spmd

    if _NC is None:
        _NC = build_nc()
    in_maps = make_in_maps(x, Wq, Wk, Wv, Wo)
    res = run_bass_kernel_spmd(_NC, in_maps, core_ids=list(range(NCORES)))
    y = res.results[0]["y"].astype(np.float32)
    for c in range(1, NCORES):
        y = y + res.results[c]["y"]
    return y.reshape(B, T, D)


# revision 32
# speedup vs baseline: 1.0308x; 1.0308x over previous
"""Multi-head attention (B=4, T=2048, D=1024, H=16, causal) on 8 trn2 cores.

Sharding: 2 heads per core (tensor-parallel on H). Each core computes
q/k/v projections for its 128-row slice of Wq/Wk/Wv over all tokens,
causal attention for its 2 heads over all 4 batches, and a partial
o_proj contribution y_c = A_c @ Wo[:, slice].T.  The host sums the 8
partial outputs (the "all-reduce after o_proj" of the sharding hint).

Layout strategy: activations live transposed ([feature, token]) on
device so the matmul contraction dim is always the partition dim:
  qT/kT/vT [128=2*64, T]  <- W-slice-T tiles @ xT chunks
  scoresT  [128 k-tok, 512 q-tok] <- kT-tile.T @ qT   (per head)
  softmax: no max-subtraction (logits are O(+-8); exp is safe in f32);
           exp on ACT; causal masking multiplies diagonal tiles with a
           precomputed 0/1 band; row sums come free as a column of
           ones appended to v; normalization is deferred to after
           attn@v: broadcast the denominators across partitions with
           K=1 PE outer products, reciprocal_approx on the broadcast.
  aT       [65, 512] PSUM accum over k-tiles (row 64 = softmax denom)
  y        [tok, D] partial, via lhsT=aT tiles (natural output layout).
Matmuls run in bf16 (1 cyc/row on the PE; f32r measured 2 cyc/row).
"""

import sys

sys.path.insert(0, "/opt/trn_rl_repo")

import ml_dtypes
import numpy as np

import concourse.bass as bass
import concourse.mybir as mybir
import concourse.tile as tile
from concourse.masks import make_identity

F32 = mybir.dt.float32
F32R = mybir.dt.float32r
BF16 = mybir.dt.bfloat16
EXP = mybir.ActivationFunctionType.Exp

B, T, D, H = 4, 2048, 1024, 16
DH = D // H
NCORES = 8
HPC = H // NCORES          # heads per core (2)
HS = HPC * DH              # head-slice width per core (128)
QC = 512                   # q-tokens per chunk (PSUM free-dim limit, f32)
KT = 128                   # k-tokens per tile (partition dim)

_wsplit_n = [0]


def split_embedded_waits(nc):
    """Move embedded on_wait conditions into standalone EventSemaphore
    instructions.  The walrus build in this container rejects embedded
    sync waits on compute instruction structs ("Too many sync wait
    commands"); raw-bass-style standalone waits compile fine."""
    moved = 0
    for func in nc.m.functions:
        for blk in func.blocks:
            insts = list(blk.instructions)
            out = []
            changed = False
            for ins in insts:
                si = ins.sync_info
                waits = list(si.on_wait) if (si is not None and si.on_wait) else []
                limit = 1 if ins.opcode in ("EventSemaphore", "Drain") else 0
                if len(waits) > limit:
                    head = waits[:-limit] if limit else waits
                    tail = waits[-limit:] if limit else []
                    for w in head:
                        _wsplit_n[0] += 1
                        out.append(
                            mybir.InstEventSemaphore(
                                name=f"I-wsplit-{_wsplit_n[0]}",
                                engine=ins.engine,
                                sync_info=mybir.SyncInfo(on_wait=[w], on_update=[]),
                            )
                        )
                        moved += 1
                    ins.sync_info = mybir.SyncInfo(
                        on_wait=list(tail),
                        on_update=list(si.on_update) if si.on_update else [],
                    )
                    changed = True
                out.append(ins)
            if changed:
                blk.instructions = out
    return moved


def build_nc(nb=B, t=T, split_waits=True):
    """Build the per-core Bass program. nb/t shrinkable for simulation.
    split_waits must be True for hardware (walrus); False for CoreSim."""
    nqc = t // QC            # q-chunks per batch
    nkt = t // KT            # k-tiles per batch
    ntok = nb * t

    nc = bass.Bass("TRN2", target_bir_lowering=False)

    xT_d = nc.dram_tensor("xT", [D, ntok], BF16, kind="ExternalInput")
    wq_d = nc.dram_tensor("wq", [D, HS], BF16, kind="ExternalInput")
    wk_d = nc.dram_tensor("wk", [D, HS], BF16, kind="ExternalInput")
    wv_d = nc.dram_tensor("wv", [D, HS], BF16, kind="ExternalInput")
    wo_d = nc.dram_tensor("wo", [HS, D], BF16, kind="ExternalInput")
    tri_d = nc.dram_tensor("tri", [KT, KT], BF16, kind="ExternalInput")
    y_d = nc.dram_tensor("y", [ntok, D], F32, kind="ExternalOutput")

    nd = D // 128            # 8 d-in tiles

    with tile.TileContext(nc) as tc:
        with (
            tc.tile_pool(name="wpool", bufs=1) as wpool,
            tc.tile_pool(name="xin", bufs=2) as xin,
            tc.tile_pool(name="qkv", bufs=2) as qkvp,
            tc.tile_pool(name="vaug", bufs=2) as vaugp,
            tc.tile_pool(name="exps", bufs=4) as expp,
            tc.tile_pool(name="misc", bufs=2) as miscp,
            tc.tile_pool(name="yout", bufs=3) as youtp,
            tc.tile_pool(name="psc", bufs=2, space="PSUM") as psc,
            tc.tile_pool(name="pa", bufs=1, space="PSUM") as pa,
            tc.tile_pool(name="pm", bufs=2, space="PSUM") as pm,
        ):
            # resident weights
            wq_sb = wpool.tile([128, nd, HS], BF16, tag="wq")
            wk_sb = wpool.tile([128, nd, HS], BF16, tag="wk")
            wv_sb = wpool.tile([128, nd, HS], BF16, tag="wv")
            for wd, ws in ((wq_d, wq_sb), (wk_d, wk_sb), (wv_d, wv_sb)):
                nc.sync.dma_start(ws[:], wd[:].rearrange("(a p) m -> p a m", p=128))
            wo_sb = wpool.tile([HS, D], BF16, tag="wo")
            nc.sync.dma_start(wo_sb[:], wo_d[:])
            tri_sb = wpool.tile([KT, KT], BF16, tag="tri")
            nc.sync.dma_start(tri_sb[:], tri_d[:])
            # constants: f32 masters, rounded into matmul dtypes via copies
            ones_f = wpool.tile([128, max(nkt, 128)], F32, tag="ones_f")
            nc.vector.memset(ones_f[:], 1.0)
            zeros_f = wpool.tile([1, 64], F32, tag="zeros_f")
            nc.vector.memset(zeros_f[:], 0.0)
            # head-expander rows: e0row = [1]*64+[0]*64, e1row = [0]*64+[1]*64
            e0row = wpool.tile([1, 128], F32R, tag="e0row")
            e1row = wpool.tile([1, 128], F32R, tag="e1row")
            nc.vector.tensor_copy(e0row[:, 0:64], ones_f[0:1, 0:64])
            nc.vector.tensor_copy(e0row[:, 64:128], zeros_f[:])
            nc.vector.tensor_copy(e1row[:, 0:64], zeros_f[:])
            nc.vector.tensor_copy(e1row[:, 64:128], ones_f[0:1, 0:64])
            ident = wpool.tile([128, 128], F32, tag="ident")
            make_identity(nc, ident[:])

            class OproJob:
                """Deferred o_proj for one 512-token chunk: 8 matmul+copy
                pieces stepped one at a time between k-iterations, then one
                fused DMA of the [512, D] result."""

                def __init__(self, row0, aT):
                    self.row0, self.aT = row0, aT
                    self.ysb = youtp.tile([128, QC // 128, D], F32, tag="ysb")
                    self.pieces = [
                        (tt, do)
                        for tt in range(QC // 128)
                        for do in range(D // 512)
                    ]
                    self.i = 0

                def step(self):
                    tt, do = self.pieces[self.i]
                    self.i += 1
                    yp = pm.tile([128, 512], F32, tag="m", bufs=1)
                    nc.tensor.matmul(
                        yp[:],
                        self.aT[:, 128 * tt : 128 * (tt + 1)],
                        wo_sb[:, 512 * do : 512 * (do + 1)],
                        start=True,
                        stop=True,
                    )
                    nc.vector.tensor_copy(
                        self.ysb[:, tt, 512 * do : 512 * (do + 1)], yp[:]
                    )
                    if self.i == len(self.pieces):
                        nc.sync.dma_start(
                            y_d[self.row0 : self.row0 + QC, :].rearrange(
                                "(a p) n -> p a n", p=128
                            ),
                            self.ysb[:],
                        )
                        return False
                    return True

            ojob = [None]

            def load_xt(b, nsplit=1):
                base = b * t
                xt = xin.tile([128, nd, t], BF16, tag="xt")
                step = t // nsplit
                for s in range(nsplit):
                    nc.sync.dma_start(
                        xt[:, :, s * step : (s + 1) * step],
                        xT_d[:, base + s * step : base + (s + 1) * step].rearrange(
                            "(a p) n -> p a n", p=128
                        ),
                    )
                return xt

            

            xt_next = load_xt(0, nsplit=4)
            for b in range(nb):
                base = b * t
                xt_b = xt_next
                # ---- q/k/v projections for batch b ----
                # kT is stored zero-padded per head (kT0z rows 0:64 = head0,
                # rows 64:128 = 0; kT1z the reverse) so the scores matmul can
                # stream the full 128-partition qT at full SBUF rate.
                qT = qkvp.tile([128, t], BF16, tag="qT")
                kT0z = qkvp.tile([128, t], BF16, tag="kT0z")
                kT1z = qkvp.tile([128, t], BF16, tag="kT1z")
                nc.vector.memset(kT0z[64:128, :], 0.0)
                nc.vector.memset(kT1z[0:64, :], 0.0)
                vT = qkvp.tile([128, t], F32, tag="vT")
                for ch in range(nqc):
                    cs = slice(QC * ch, QC * (ch + 1))
                    for wi, ws in enumerate((wq_sb, wk_sb, wv_sb)):
                        ps = psc.tile([128, 2, QC], F32, tag="sc")
                        for kd in range(nd):
                            nc.tensor.matmul(
                                ps[:, 0, :],
                                ws[:, kd, :],
                                xt_b[:, kd, cs],
                                start=(kd == 0),
                                stop=(kd == nd - 1),
                            )
                        if wi == 0:
                            nc.vector.tensor_copy(qT[:, cs], ps[:, 0, :])
                        elif wi == 1:
                            nc.vector.tensor_copy(kT0z[0:64, cs], ps[0:64, 0, :])
                            nc.vector.tensor_copy(kT1z[64:128, cs], ps[64:128, 0, :])
                        else:
                            nc.vector.tensor_copy(vT[:, cs], ps[:, 0, :])

                if b + 1 < nb:
                    xt_next = load_xt(b + 1)

                # ---- transpose v into [k-tok, dh(+ones)] tiles ----
                v0 = vaugp.tile([128, nkt, DH + 1], BF16, tag="v0")
                v1 = vaugp.tile([128, nkt, DH + 1], BF16, tag="v1")
                nc.vector.tensor_copy(v0[:, :, DH : DH + 1], ones_f[:, 0:nkt])
                nc.vector.tensor_copy(v1[:, :, DH : DH + 1], ones_f[:, 0:nkt])
                for kt in range(nkt):
                    tp = pm.tile([128, 512], F32, tag="rb", bufs=1)
                    nc.tensor.transpose(
                        tp[:, 0:128], vT[:, KT * kt : KT * (kt + 1)], ident[:]
                    )
                    nc.vector.tensor_copy(v0[:, kt, 0:DH], tp[:, 0:DH])
                    nc.vector.tensor_copy(v1[:, kt, 0:DH], tp[:, DH : 2 * DH])

                # ---- attention + o_proj per q-chunk ----
                # k-loop emitted software-pipelined (scores two steps ahead
                # of attn@v); the previous chunk's o_proj matmuls and output
                # copies are drip-fed between k-iterations so the PE never
                # sits in a blocked o_proj stretch, and normalization is a
                # GPSIMD divide by the PE-broadcast denominators (no
                # reciprocal, nothing heavy on the DVE critical path).
                for qc in range(nqc):
                    q0 = QC * qc
                    apair = pa.tile([DH + 1, 2, QC], F32, tag="apair")
                    hi = qc * (QC // KT) + (QC // KT)  # causal: k-tiles 0..hi-1

                    def emit_scores(kt):
                        scp = psc.tile([128, 2, QC], F32, tag="sc")
                        nc.tensor.matmul(
                            scp[:, 0, :],
                            kT0z[:, KT * kt : KT * (kt + 1)],
                            qT[:, q0 : q0 + QC],
                            start=True,
                            stop=True,
                        )
                        nc.tensor.matmul(
                            scp[:, 1, :],
                            kT1z[:, KT * kt : KT * (kt + 1)],
                            qT[:, q0 : q0 + QC],
                            start=True,
                            stop=True,
                        )
                        return scp

                    def emit_tail(kt, scp):
                        ep = expp.tile([128, 2, QC], BF16, tag="ep")
                        off = KT * kt - q0
                        if off < 0:
                            nc.scalar.activation(ep[:], scp[:], EXP)
                        else:
                            # diagonal tile: exp only the causally valid
                            # span, zero the rest, apply the 0/1 band
                            for h in (0, 1):
                                nc.scalar.activation(
                                    ep[:, h, off:QC], scp[:, h, off:QC], EXP
                                )
                                if off > 0:
                                    nc.vector.memset(ep[:, h, 0:off], 0.0)
                                nc.vector.tensor_mul(
                                    ep[:, h, off : off + KT],
                                    ep[:, h, off : off + KT],
                                    tri_sb[:],
                                )
                        for h, vh in ((0, v0), (1, v1)):
                            nc.tensor.matmul(
                                apair[:, h, :],
                                vh[:, kt, :],
                                ep[:, h, :],
                                start=(kt == 0),
                                stop=(kt == hi - 1),
                                skip_group_check=True,
                            )

                    pend = [emit_scores(0)]
                    if hi > 1:
                        pend.append(emit_scores(1))
                    for kt in range(2, hi):
                        emit_tail(kt - 2, pend.pop(0))
                        pend.append(emit_scores(kt))
                        if ojob[0] is not None and not ojob[0].step():
                            ojob[0] = None
                    for j, scp in enumerate(pend):
                        emit_tail(hi - len(pend) + j, scp)
                    while ojob[0] is not None:
                        if not ojob[0].step():
                            ojob[0] = None

                    # free apair fast: pull out the two heads + denominators
                    sums01 = miscp.tile([1, 2, QC], F32R, tag="sums01")
                    nc.vector.tensor_copy(sums01[:], apair[DH : DH + 1, :, :])
                    aT = qkvp.tile([128, QC], BF16, tag="aT", bufs=3)
                    nc.scalar.copy(aT[0:DH, :], apair[0:DH, 0, :])
                    nc.scalar.copy(aT[DH : 2 * DH, :], apair[0:DH, 1, :])
                    # broadcast denominators across partitions (K=1 PE outer
                    # products), reciprocal on the broadcast, scale, queue o_proj
                    rb = pm.tile([128, 512], F32, tag="rb", bufs=1)
                    nc.tensor.matmul(
                        rb[:, 0:QC], e0row[:], sums01[:, 0, :],
                        start=True, stop=False, skip_group_check=True,
                    )
                    nc.tensor.matmul(
                        rb[:, 0:QC], e1row[:], sums01[:, 1, :],
                        start=False, stop=True, skip_group_check=True,
                    )
                    rcp = miscp.tile([128, QC], F32, tag="rcp")
                    nc.vector.reciprocal(rcp[:], rb[:, 0:QC])
                    nc.vector.tensor_mul(aT[:], aT[:], rcp[:])
                    ojob[0] = OproJob(base + q0, aT)
            while ojob[0] is not None:
                if not ojob[0].step():
                    ojob[0] = None

    if split_waits:
        split_embedded_waits(nc)
    return nc


def make_tri():
    tri = np.zeros((KT, KT), np.float32)
    j = np.arange(KT)[None, :]
    k = np.arange(KT)[:, None]
    tri[j >= k] = 1.0
    return tri.astype(ml_dtypes.bfloat16)


def make_in_maps(x, Wq, Wk, Wv, Wo):
    ntok = x.shape[0] * x.shape[1]
    bf = ml_dtypes.bfloat16
    xT = np.ascontiguousarray(x.reshape(ntok, D).T).astype(bf)
    tri = make_tri()
    scale = np.float32(1.0 / np.sqrt(DH))
    in_maps = []
    for c in range(NCORES):
        hs = slice(HS * c, HS * (c + 1))
        in_maps.append(
            {
                "xT": xT,
                "wq": np.ascontiguousarray((Wq[hs, :] * scale).T).astype(bf),
                "wk": np.ascontiguousarray(Wk[hs, :].T).astype(bf),
                "wv": np.ascontiguousarray(Wv[hs, :].T).astype(bf),
                "wo": np.ascontiguousarray(Wo[:, hs].T).astype(bf),
                "tri": tri,
            }
        )
    return in_maps


_NC = None


def kernel(**inputs):
    global _NC
    x = np.asarray(inputs["x"], np.float32)
    Wq = np.asarray(inputs["Wq"], np.float32)
    Wk = np.asarray(inputs["Wk"], np.float32)
    Wv = np.asarray(inputs["Wv"], np.float32)
    Wo = np.asarray(inputs["Wo"], np.float32)

    from concourse.bass_utils import run_bass_kernel_spmd

    if _NC is None:
        _NC = build_nc()
    in_maps = make_in_maps(x, Wq, Wk, Wv, Wo)
    res = run_bass_kernel_spmd(_NC, in_maps, core_ids=list(range(NCORES)))
    y = res.results[0]["y"].astype(np.float32)
    for c in range(1, NCORES):
        y = y + res.results[c]["y"]
    return y.reshape(B, T, D)
